# revision 7
# baseline (speedup 1.0000x reference)
"""Trainium2 Bass kernel for nn_BertWordPair (ragged RoPE pair scores).

Strategy
--------
Inputs: qw, kw (B=8, S=768, H=4, D=256) fp32; token_index, thread_id (S,) int32.
Output: (B, S, S, H) fp32 where each (row-block, col-block) pair of the 6x128
thread-block grid uses one of three RoPE sign regimes:
    pp: rope(q,+pos) . rope(k,+pos)
    np: rope(q,-pos) . rope(k,+pos)   (0 < ti_r < ti_c)
    pn: rope(q,+pos) . rope(k,-pos)   (ti_c > 0, ti_r > ti_c)

Host side precomputes the rotated variants qp, qn, kp in a de-interleaved
(pair-index, token) layout, casts to fp16, and shards batch across the 8
cores (1 dialogue per core). kn is derived on-device from kp by a DVE
fp16 rotation (kn = R(-2θ)kp) using one small per-block-pattern
[c2|s2|-s2|c2] table broadcast across heads — two fused DVE ops per
128-block — saving its DMA. Device work: matmuls (one 128x128x256
contraction per output block/head, fp16 in, fp32 PSUM, 4 heads packed
per PSUM bank), one head-interleaving fp32->fp16 PSUM->SBUF copy per
bank (spread across ACT/Pool, DVE joining once the rotation ladder
drains), and half-row fp16 output DMAs (the host upcasts to fp32).
The SP DMA ring carries ~9.0MB per core (qp+qn+kp in, fp16 scores out),
ordered so row 0's dependencies land early and the ring never stalls;
the tiny table rides the Pool SWDGE queue off the critical ring.
"""

import os

import numpy as np

ROPE_BASE = 10000.0
B, S, H, D = 8, 768, 4, 256
HALF = D // 2  # 128
BLK = 128
NB = S // BLK  # 6
N_CORES = 8

_prog_cache = {}


def _host_rotations(qw, kw, token_index):
    """Return u/v (even/odd) rotated variants, fp32.

    Shapes: (B, S, H, HALF) each for (qp_u, qp_v, qn_u, qn_v, kp_u, kp_v,
    kn_u, kn_v)."""
    inv_freq = np.power(
        np.float32(ROPE_BASE),
        (np.arange(HALF, dtype=np.float32) * np.float32(-2.0 / D)),
    )  # (HALF,)
    pos = token_index.astype(np.float32)  # (S,)
    theta = pos[:, None] * inv_freq[None, :]  # (S, HALF)
    cos = np.cos(theta)[None, :, None, :]  # (1,S,1,HALF)
    sin = np.sin(theta)[None, :, None, :]

    out = []
    for x in (qw, kw):
        u = x[..., 0::2]  # (B,S,H,HALF)
        v = x[..., 1::2]
        uc = u * cos
        vs = v * sin
        vc = v * cos
        us = u * sin
        # positive rotation
        out.append((uc - vs, vc + us))
        # negative rotation (sin -> -sin)
        out.append((uc + vs, vc - us))
    return out  # [(qp_u,qp_v),(qn_u,qn_v),(kp_u,kp_v),(kn_u,kn_v)]


def _to_device_layout(u, v, blocks):
    """(B,S,H,HALF) u/v -> (B, H, 2, HALF, T) fp16 for the given token blocks."""
    cols = np.concatenate([np.arange(b * BLK, (b + 1) * BLK) for b in blocks])
    u = u[:, cols]  # (B,T,H,HALF)
    v = v[:, cols]
    arr = np.stack([u, v], axis=2)  # (B,T,2,H,HALF)
    arr = np.transpose(arr, (0, 3, 2, 4, 1))  # (B,H,2,HALF,T)
    return np.ascontiguousarray(arr.astype(np.float16))


def _plan(token_index, thread_id):
    """Build the static execution plan, or None if the structure doesn't
    match the fast path (aligned 128-blocks, shared per-block position
    pattern)."""
    tid = np.asarray(thread_id)
    tok = np.asarray(token_index)
    if tid.shape != (S,) or tok.shape != (S,):
        return None
    blocks = tid.reshape(NB, BLK)
    if not np.all(blocks == blocks[:, :1]):
        return None  # thread blocks not aligned to the 128 grid
    # all blocks must share one position pattern (the rotation table is
    # built once and reused for every kn block)
    pat = tok.reshape(NB, BLK)
    if not np.all(pat == pat[:1]):
        return None
    tvals = blocks[:, 0]
    regimes = []
    for i in range(NB):
        row = []
        for j in range(NB):
            ti_r, ti_c = tvals[i], tvals[j]
            if ti_r > 0 and ti_r < ti_c:
                row.append("np")
            elif ti_c > 0 and ti_r > ti_c:
                row.append("pn")
            else:
                row.append("pp")
        regimes.append(row)

    qn_blocks = sorted(
        {i for i in range(NB) if any(regimes[i][j] == "np" for j in range(NB))}
    )
    kn_blocks = sorted(
        {j for j in range(NB) if any(regimes[i][j] == "pn" for i in range(NB))}
    )
    return {
        "regimes": tuple(tuple(r) for r in regimes),
        "qn_blocks": tuple(qn_blocks),
        "kn_blocks": tuple(kn_blocks),
    }


def _build_program(plan):
    import concourse.bass as bass  # noqa: F401
    import concourse.tile as tile
    from concourse import bacc, mybir

    f16 = mybir.dt.float16
    f32 = mybir.dt.float32

    regimes = plan["regimes"]
    qn_blocks = list(plan["qn_blocks"])
    kn_blocks = list(plan["kn_blocks"])
    nqn = max(1, len(qn_blocks))
    qn_pos = {b: idx for idx, b in enumerate(qn_blocks)}
    TQ = nqn * BLK

    nc = bacc.Bacc(None, target_bir_lowering=False)
    qp_d = nc.dram_tensor("qp", [H, 2, HALF, S], f16, kind="ExternalInput")
    qn_d = nc.dram_tensor("qn", [H, 2, HALF, TQ], f16, kind="ExternalInput")
    kp_d = nc.dram_tensor("kp", [H, 2, HALF, S], f16, kind="ExternalInput")
    # [c2|s2|-s2|c2] over one 128-token block pattern: the (ab, c) view with
    # both strides = BLK gives [c2|s2] at ab=0 and [-s2|c2] at ab=1, so one
    # broadcast mul + one dual-add produce kn_e = e*c2+o*s2 (ab=0 sum) and
    # kn_o = o*c2-e*s2 (ab=1 sum) for all 4 heads at once.
    kt_d = nc.dram_tensor("kt", [HALF, 4 * BLK], f16, kind="ExternalInput")
    out_d = nc.dram_tensor("out", [S, S, H], f16, kind="ExternalOutput")

    with tile.TileContext(nc) as tc:
        with (
            tc.tile_pool(name="inp", bufs=1) as inp,
            tc.tile_pool(name="psum", bufs=7, space="PSUM") as pp,
            tc.tile_pool(name="warm", bufs=1, space="PSUM") as wp,
            tc.tile_pool(name="stage", bufs=3) as stp,
            tc.tile_pool(name="rtmp", bufs=2) as rtmp,
        ):
            # Input tiles: (128 partitions = pair index, H*2*T tokens) fp16.
            qp_t = inp.tile([HALF, H * 2 * S], f16, tag="qp")
            qn_t = inp.tile([HALF, H * 2 * TQ], f16, tag="qn")
            kp_t = inp.tile([HALF, H * 2 * S], f16, tag="kp")
            kn_t = inp.tile([HALF, H * 2 * S], f16, tag="kn")
            kt_t = inp.tile([HALF, 4 * BLK], f16, tag="kt")

            qp_v = qp_t[:].rearrange("p (h c t) -> p h c t", h=H, c=2, t=S)
            kp_v = kp_t[:].rearrange("p (h c t) -> p h c t", h=H, c=2, t=S)
            qn_v = qn_t[:].rearrange("p (h c t) -> p h c t", h=H, c=2, t=TQ)
            qp_dv = qp_d[:].rearrange("h c p t -> p h c t")
            kp_dv = kp_d[:].rearrange("h c p t -> p h c t")
            qn_dv = qn_d[:].rearrange("h c p t -> p h c t")

            # The tiny rotation table rides the Pool SWDGE queue so the SP
            # ring starts straight into the big transfers.
            nc.gpsimd.dma_start(kt_t[:], kt_d[:])
            # SP ring input order: q/k first chunks, row-0's lhsT second
            # chunk (blocks 0-1), all of kp's second chunk (row 0 needs every
            # kp block; the kn ladder starts here too), then qn, then the
            # remaining qp second chunks (rows 2-5 lhsT, needed one output
            # row-slot at a time).
            nc.sync.dma_start(kp_v[:, :, 0], kp_dv[:, :, 0])
            nc.sync.dma_start(qp_v[:, :, 0], qp_dv[:, :, 0])
            nc.sync.dma_start(
                qp_v[:, :, 1, 0 : 2 * BLK], qp_dv[:, :, 1, 0 : 2 * BLK]
            )
            nc.sync.dma_start(kp_v[:, :, 1], kp_dv[:, :, 1])
            nc.sync.dma_start(qn_v[:], qn_dv[:])
            nc.sync.dma_start(
                qp_v[:, :, 1, 2 * BLK : S], qp_dv[:, :, 1, 2 * BLK : S]
            )

            # PE reaches full clock only after ~3us of continuous execution,
            # and any idle gap resets the ramp. Warmup matmuls into a scratch
            # PSUM bank keep PE hot through the natural stall windows so the
            # real matmuls run at full speed: phase A before the first inputs
            # land, phase B while kp's second chunk streams, phase C while
            # row 1 waits for the qn DMA.
            wtile = inp.tile([HALF, BLK], f16, tag="wtile")
            nc.vector.memset(wtile[:], 0.0)
            wbank = wp.tile([BLK, BLK], f32, tag="wbank")

            def emit_warmup(n):
                for _ in range(n):
                    nc.tensor.matmul(
                        wbank[:], wtile[:], wtile[:], start=True, stop=True
                    )

            emit_warmup(105)

            # kn = R(-2θ) kp per derived 128-block, all 4 heads fused via
            # stride-0 broadcasts: txy[ab,h,c,t] = kp[h,c,t] * tab[ab,c,t],
            # then kn[c'=ab] = sum over c of txy.
            tab = (
                kt_t[:]
                .rearrange("p (ab c t) -> p ab c t", ab=2, c=2)
                .unsqueeze(2)
                .broadcast_to([HALF, 2, H, 2, BLK])
            )

            def emit_rotation(b):
                pepo = (
                    kp_v[:, :, :, b * BLK : (b + 1) * BLK]
                    .unsqueeze(1)
                    .broadcast_to([HALF, 2, H, 2, BLK])
                )
                txy = rtmp.tile([HALF, 2 * H * 2 * BLK], f16, tag="txy")
                txy_v = txy[:].rearrange(
                    "p (ab h c t) -> p ab h c t", ab=2, h=H, c=2, t=BLK
                )
                nc.vector.tensor_mul(txy_v, pepo, tab)
                dst = kn_t[:].rearrange(
                    "p (h c t) -> p c h t", h=H, c=2, t=S
                )[:, :, :, b * BLK : (b + 1) * BLK]
                nc.vector.tensor_add(
                    dst, txy_v[:, :, :, 0, :], txy_v[:, :, :, 1, :]
                )

            def lhs_slice(reg, h, c, blk):
                if reg == "np":
                    return qn_t[:, (h * 2 + c) * TQ + qn_pos[blk] * BLK :][
                        :, :BLK
                    ]
                return qp_t[:, (h * 2 + c) * S + blk * BLK :][:, :BLK]

            def rhs_slice(reg, h, c, blk):
                t = kn_t if reg == "pn" else kp_t
                return t[:, (h * 2 + c) * S + blk * BLK :][:, :BLK]

            # Evacuation engine per (row, bank): ACT + Pool carry the early
            # rows; DVE joins once the kn rotation ladder has drained.
            evac_plan = {
                0: "APAPAP",
                1: "PAAPAA",
                2: "APAPAA",
                3: "APVAPA",
                4: "VAPVAP",
                5: "VAVPAV",
            }

            def emit_row(i, stage, warm_before_c1=0, warm_before_var=0):
                banks = {}
                order = sorted(
                    range(NB), key=lambda j: (regimes[i][j] != "pp", j)
                )
                for c in range(2):
                    if c == 1:
                        emit_warmup(warm_before_c1)
                    warmed_var = False
                    for j in order:
                        reg = regimes[i][j]
                        if c == 0 and reg != "pp" and not warmed_var:
                            warmed_var = True
                            emit_warmup(warm_before_var)
                        if c == 0:
                            bank = pp.tile([BLK, BLK * H], f32, tag="bank")
                            banks[j] = bank
                        bank = banks[j]
                        for h in range(H):
                            nc.tensor.matmul(
                                bank[:, h * BLK : (h + 1) * BLK],
                                lhs_slice(reg, h, c, i),
                                rhs_slice(reg, h, c, j),
                                start=(c == 0 and h == 0),
                                stop=(c == 1 and h == H - 1),
                            )
                for j in range(NB):
                    bank = banks[j]
                    # one head-interleaving evacuation copy per bank:
                    # bank (p, (h n)) -> stage (p, (n h)) at block j
                    dst_blk = stage[:, j * (BLK * H) : (j + 1) * (BLK * H)]
                    dst_blk = dst_blk.rearrange("p (n h) -> p h n", h=H)
                    src_blk = bank[:].rearrange("p (h n) -> p h n", n=BLK)
                    eng = evac_plan[i][j]
                    if eng == "V":
                        nc.vector.tensor_copy(dst_blk, src_blk)
                    elif eng == "P":
                        nc.gpsimd.tensor_copy(dst_blk, src_blk)
                    else:
                        nc.scalar.copy(dst_blk, src_blk)
                # Two half-row output DMAs so the stream isn't gated on the
                # whole row's evacuation.
                HW2 = NB // 2 * BLK * H
                nc.sync.dma_start(
                    out_d[i * BLK : (i + 1) * BLK, 0 : S // 2].rearrange(
                        "p n h -> p (n h)"
                    ),
                    stage[:, 0:HW2],
                )
                nc.sync.dma_start(
                    out_d[i * BLK : (i + 1) * BLK, S // 2 : S].rearrange(
                        "p n h -> p (n h)"
                    ),
                    stage[:, HW2 : 2 * HW2],
                )

            for i in range(NB):
                stage = stp.tile([BLK, S * H], f16, tag="stage")
                emit_row(
                    i,
                    stage,
                    warm_before_c1=27 if i == 0 else 0,
                    warm_before_var=25 if i == 1 else 0,
                )
                # kn ladder: emitted after row 0 so DVE's queue holds the
                # rotations ahead of any row 1+ work it may pick up; each
                # block's ops only depend on kp_c1 + the table.
                if i == 0:
                    for b in kn_blocks:
                        emit_rotation(b)
    nc.finalize()
    return nc


def _reference_fallback(qw, kw, token_index, thread_id):
    """Pure numpy fallback for unexpected block structure."""
    rots = _host_rotations(qw, kw, token_index)
    (qp_u, qp_v), (qn_u, qn_v), (kp_u, kp_v), (kn_u, kn_v) = rots

    def interleave(u, v):
        x = np.empty(u.shape[:-1] + (D,), dtype=np.float32)
        x[..., 0::2] = u
        x[..., 1::2] = v
        return x

    q_p = interleave(qp_u, qp_v)
    q_n = interleave(qn_u, qn_v)
    k_p = interleave(kp_u, kp_v)
    k_n = interleave(kn_u, kn_v)
    s_pp = np.einsum("bmhd,bnhd->bmnh", q_p, k_p)
    s_np = np.einsum("bmhd,bnhd->bmnh", q_n, k_p)
    s_pn = np.einsum("bmhd,bnhd->bmnh", q_p, k_n)
    ti_r = thread_id[:, None]
    ti_c = thread_id[None, :]
    sx = ((ti_r > 0) & (ti_r < ti_c))[None, :, :, None]
    sy = ((ti_c > 0) & (ti_r > ti_c))[None, :, :, None]
    return np.where(sx, s_np, np.where(sy, s_pn, s_pp)).astype(np.float32)


def _rotation_table(token_index):
    """[c2|s2|-s2|c2] fp16 table (HALF, 4*BLK) for one block's pattern."""
    inv_freq = np.power(
        np.float32(ROPE_BASE),
        (np.arange(HALF, dtype=np.float32) * np.float32(-2.0 / D)),
    )
    theta = token_index[:BLK].astype(np.float32)[:, None] * inv_freq[None, :]
    c2 = np.cos(2.0 * theta).T  # (HALF, BLK)
    s2 = np.sin(2.0 * theta).T
    return np.ascontiguousarray(
        np.concatenate([c2, s2, -s2, c2], axis=1).astype(np.float16)
    )


def kernel(qw, kw, token_index, thread_id):
    qw = np.asarray(qw, dtype=np.float32)
    kw = np.asarray(kw, dtype=np.float32)
    token_index = np.asarray(token_index)
    thread_id = np.asarray(thread_id)

    plan = _plan(token_index, thread_id)
    if plan is None or qw.shape != (B, S, H, D) or kw.shape != (B, S, H, D):
        return _reference_fallback(qw, kw, token_index, thread_id)

    rots = _host_rotations(qw, kw, token_index)
    (qp_u, qp_v), (qn_u, qn_v), (kp_u, kp_v), _ = rots
    all_blocks = list(range(NB))
    qn_blocks = list(plan["qn_blocks"]) or [0]
    qp_a = _to_device_layout(qp_u, qp_v, all_blocks)  # (B,H,2,HALF,S)
    qn_a = _to_device_layout(qn_u, qn_v, qn_blocks)
    kp_a = _to_device_layout(kp_u, kp_v, all_blocks)
    kt_a = _rotation_table(token_index)

    key = plan["regimes"]
    if key not in _prog_cache:
        _prog_cache[key] = _build_program(plan)
    nc = _prog_cache[key]

    from concourse.bass_utils import run_bass_kernel_spmd

    in_maps = [
        {"qp": qp_a[b], "qn": qn_a[b], "kp": kp_a[b], "kt": kt_a}
        for b in range(B)
    ]
    trace = bool(int(os.environ.get("KERNEL_TRACE", "0")))
    res = None
    for attempt in range(3):
        try:
            res = run_bass_kernel_spmd(
                nc,
                in_maps,
                core_ids=list(range(N_CORES)),
                trace=trace,
            )
            break
        except Exception:
            # transient NRT/device blips (e.g. NRT_EXEC_UNIT_UNRECOVERABLE)
            # have been observed on otherwise-correct programs; retry.
            if attempt == 2:
                raise
    if res.exec_time_ns is not None:
        print(f"HW exec time: {res.exec_time_ns} ns")
    if res.instructions_and_trace is not None:
        print(f"trace: {res.instructions_and_trace[1]}")

    out = np.stack([res.results[b]["out"] for b in range(B)], axis=0)
    return out.astype(np.float32)
